# revision 20
# baseline (speedup 1.0000x reference)
"""BinaryConv2d (3x3, 64->64, SAME, binarized +-1 weights, bias+ReLU) on 8 TRN2 cores.

Strategy (data-parallel, 2 images per core):
  - Host: quantize W to +-1; split x into fp8 e4m3 hi/lo residual planes
    (hi = e4m3(x), lo = e4m3(x - hi); hi+lo ~ x to ~2^-8 rel) laid out
    channel-major with row-pair interleave + H/W zero padding, so the device
    needs only plain (non-transpose) full-rate DMAs.
  - Device per core, per image: 56 PSUM groups, each = 2 output row-pairs
    (4 rows x 224 px). Per group 6 DoubleRow fp8 matmuls (3 kw taps x a/b
    row-block), each contracting hi+lo planes in one pass (K=2x128,
    N=448 moving cols), accumulating in PSUM fp32; ScalarE bias+ReLU+fp16
    into a channel-major slab; one contiguous 16-pair store per slab, issued
    from the otherwise-idle GPSIMD queue (SWDGE) so store issuance never
    contends with activations (ACT) or input loads (SP).
  - Host: un-permute channel-major fp16 output to NHWC fp32.

DRAM layouts (per core):
  xi [2, 128, 2, 114, 226] fp8   xi[i, (r c), q, j, 1+w] = plane_q of
                                 xpad[i, 2j+r-1, w, c]  (rows -1/224/225 and
                                 w cols 0, 225 are zero)
  wm [128, 6, 2, 128] fp8        blocks a/dx0..2, b/dx0..2; both K-tile
                                 slots hold the same +-1 block
  bv [128, 1] fp32               bias replicated to both row halves
  y  [2, 128, 112, 224] fp16     y[i, (r c), k, w] = out[i, 2k+r, w, c]
"""

import os

import numpy as np

import concourse.mybir as mybir
import concourse.tile as tile
from concourse import bacc
from concourse.bass_utils import run_bass_kernel_spmd

NCORES = 8
IMG = 2
H = W = 224
C = 64
NPB = 114          # padded row-pairs per image (113 real + 1 zero)
WPX = 226          # per-row width in px cols (1 zero + 224 data + 1 zero)
NPAIR = 112        # output row pairs per image
GI = 15            # row-pairs per input DMA chunk
PG = 16            # output pairs per store slab (8 PSUM groups)
F8 = mybir.dt.float8e4
F16 = mybir.dt.float16
F32 = mybir.dt.float32

USE_FP8 = os.environ.get("KERNEL_FP8") is not None
# 3-pass structural DoubleRow: Ko plane 0 = row-pair j with weight block a,
# Ko plane 1 = row-pair j+1 (duplicated in DRAM) with block b, e3m4 input.
# Disabled: walrus's BIR verifier rejects DoubleRow with float8e3 (NCC_IBIR322)
# — the interleave datapath is fp8e4/e5 only — and e4m3 single-plane error
# (2.7%) exceeds the 2e-2 gate.
STRUCT = os.environ.get("KERNEL_STRUCT") is not None
F8E3 = mybir.dt.float8e3
if STRUCT and F8E3 not in mybir.MATMUL_PERF_MODE_DTYPES:
    mybir.MATMUL_PERF_MODE_DTYPES = mybir.MATMUL_PERF_MODE_DTYPES + (F8E3,)

_cached = None


def _dedup_ldweights(nc):
    """Remove back-to-back InstLdweights that reload identical weights (the
    PE array keeps its stationary operand across matmuls). Only drops loads
    with no sync waits/updates so scheduling is unaffected."""
    import concourse.mybir as mb

    removed = 0
    for blk in nc.m.functions[0].blocks:
        last_sig = None
        keep = []
        for inst in blk.instructions:
            if isinstance(inst, mb.InstLdweights):
                si = inst.sync_info
                clean = si is None or (not si.on_wait and not si.on_update)
                sig = str(inst.ins[0])
                if sig == last_sig and clean:
                    removed += 1
                    continue
                last_sig = sig
            keep.append(inst)
        blk.instructions[:] = keep
    return removed


def _build(repeats=1, use_fp8=USE_FP8, sbufs=6, psbufs=None, act2=True,
           store_sync=False, wouter=True, gi=19, store_split=1, pg=16,
           store_pool=True, load_split=False, tail_split=4, same_w=False,
           no_store=False, no_load=False, structural=STRUCT):
    nc = bacc.Bacc("TRN2", target_bir_lowering=False, debug=False, num_devices=NCORES)
    nplane = 2 if (use_fp8 or structural) else 1
    xdt = F8E3 if structural else (F8 if use_fp8 else F16)
    nblk = 3 if structural else 6
    xi = nc.dram_tensor("xi", [IMG, 128, nplane, NPB, WPX], xdt, kind="ExternalInput")
    wm = nc.dram_tensor("wm", [128, nblk, nplane, 128], xdt, kind="ExternalInput")
    bv = nc.dram_tensor("bv", [128, 1], F32, kind="ExternalInput")
    y = nc.dram_tensor("y", [IMG, 128, NPAIR, W], F16, kind="ExternalOutput")

    PG = pg or globals()["PG"]
    if psbufs is None:
        psbufs = 4 if act2 else 8
    store_eng = nc.gpsimd if store_pool else (nc.sync if store_sync else nc.scalar)

    with tile.TileContext(nc) as tc:
        with (
            tc.tile_pool(name="wp", bufs=1) as wp,
            tc.tile_pool(name="pbp", bufs=1) as pbp,
            tc.tile_pool(name="psp", bufs=psbufs, space="PSUM") as psp,
            tc.tile_pool(name="sp", bufs=sbufs) as sp,
        ):
            # weight/bias loads ride the scalar HWDGE queue so they overlap
            # the first input chunk on the sync queue
            wt = wp.tile([128, nblk, nplane, 128], xdt, tag="wt")
            nc.scalar.dma_start(wt[:], wm[:])
            bt = wp.tile([128, 1], F32, tag="bt")
            nc.scalar.dma_start(bt[:], bv[:])

            def mms(pb, ps, t):
                if structural:
                    # 3 DoubleRow matmuls: Ko0 = pair j (block a), Ko1 =
                    # pair j+1 (block b) for psum pairs (2t, 2t+1)
                    for dx in range(3):
                        nc.tensor.matmul(
                            ps,
                            lhsT=wt[:, dx],
                            rhs=pb[:, :, 2 * t : 2 * t + 2, dx : dx + 224],
                            start=(dx == 0),
                            stop=(dx == 2),
                            perf_mode=mybir.MatmulPerfMode.DoubleRow,
                        )
                    return
                # 6 matmuls accumulating output pairs (2t, 2t+1) into ps[:,448]
                for i in range(6):
                    dx = i % 3
                    jb = 2 * t + (0 if i < 3 else 1)
                    if use_fp8:
                        nc.tensor.matmul(
                            ps,
                            lhsT=wt[:, 0] if same_w else wt[:, i],
                            rhs=pb[:, :, jb : jb + 2, dx : dx + 224],
                            start=(i == 0),
                            stop=(i == 5),
                            perf_mode=mybir.MatmulPerfMode.DoubleRow,
                        )
                    else:
                        nc.tensor.matmul(
                            ps,
                            lhsT=wt[:, i, 0],
                            rhs=pb[:, 0, jb : jb + 2, dx : dx + 224],
                            start=(i == 0),
                            stop=(i == 5),
                        )

            for _rep in range(repeats):
              pbs = []
              for img in range(IMG):
                pb = pbp.tile([128, nplane, NPB, WPX], xdt, tag=f"pb{img}")
                gi_ = gi or GI
                # small first chunk so the first matmuls start sooner
                cuts = [0, 6] if img == 0 else [0]
                while cuts[-1] < NPB:
                    cuts.append(min(cuts[-1] + gi_, NPB))
                for ci, (j0, j1) in enumerate(zip(cuts, cuts[1:])):
                    if no_load:
                        break
                    eng = nc.scalar if (load_split and ci % 2) else nc.sync
                    eng.dma_start(pb[:, :, j0:j1], xi[img, :, :, j0:j1])
                pbs.append(pb)
              for img in range(IMG):
                pb = pbs[img]
                for g in range(NPAIR // PG):
                    sb = sp.tile([128, PG, W], F16, tag="sb")
                    if wouter:
                        # weight-block-outer: one ldweights per 8 matmuls
                        # (needs _dedup_ldweights after build)
                        pss = []
                        for _ in range(PG // 4):
                            pst = psp.tile([128, 2, 512], F32, tag="ps")
                            pss.append(pst)
                        slots = [(ps[:, h, 0:448], (PG // 2) * g + 2 * v + h)
                                 for v, ps in enumerate(pss) for h in range(2)]
                        for i in range(nblk):
                            dx = i % 3
                            off = 0 if i < 3 else 1
                            for psl, t in slots:
                                jb = 2 * t + off
                                if structural:
                                    nc.tensor.matmul(
                                        psl,
                                        lhsT=wt[:, i],
                                        rhs=pb[:, :, 2 * t : 2 * t + 2, i : i + 224],
                                        start=(i == 0),
                                        stop=(i == 2),
                                        perf_mode=mybir.MatmulPerfMode.DoubleRow,
                                    )
                                elif use_fp8:
                                    nc.tensor.matmul(
                                        psl,
                                        lhsT=wt[:, i],
                                        rhs=pb[:, :, jb : jb + 2, dx : dx + 224],
                                        start=(i == 0),
                                        stop=(i == 5),
                                        perf_mode=mybir.MatmulPerfMode.DoubleRow,
                                    )
                                else:
                                    nc.tensor.matmul(
                                        psl,
                                        lhsT=wt[:, i, 0],
                                        rhs=pb[:, 0, jb : jb + 2, dx : dx + 224],
                                        start=(i == 0),
                                        stop=(i == 5),
                                    )
                        for v, ps in enumerate(pss):
                            nc.scalar.activation(
                                sb[:, 4 * v : 4 * v + 4, :],
                                ps[:, :, 0:448].rearrange(
                                    "p i (q w) -> p i q w", w=224
                                ),
                                mybir.ActivationFunctionType.Relu,
                                bias=bt[:],
                                scale=1.0,
                            )
                        store_eng.dma_start(
                            y[img, :, PG * g : PG * (g + 1), :], sb[:]
                        )
                        continue
                    if act2:
                        # 2 PSUM banks per tile, one activation per 4 pairs
                        for v in range(PG // 4):
                            ps = psp.tile([128, 2, 512], F32, tag="ps")
                            for h in range(2):
                                t = (PG // 2) * g + 2 * v + h
                                mms(pb, ps[:, h, 0:448], t)
                            nc.scalar.activation(
                                sb[:, 4 * v : 4 * v + 4, :],
                                ps[:, :, 0:448].rearrange(
                                    "p i (q w) -> p i q w", w=224
                                ),
                                mybir.ActivationFunctionType.Relu,
                                bias=bt[:],
                                scale=1.0,
                            )
                    else:
                        for u in range(PG // 2):
                            t = (PG // 2) * g + u
                            ps = psp.tile([128, 448], F32, tag="ps")
                            mms(pb, ps[:], t)
                            nc.scalar.activation(
                                sb[:, 2 * u : 2 * u + 2, :],
                                ps[:].rearrange("p (q w) -> p q w", w=224),
                                mybir.ActivationFunctionType.Relu,
                                bias=bt[:],
                                scale=1.0,
                            )
                    # split the final slab's store so the tail drains earlier
                    nsplit = store_split if not (
                        img == IMG - 1 and g == NPAIR // PG - 1
                    ) else max(store_split, tail_split)
                    for sp_i in range(nsplit):
                        if no_store:
                            break
                        w0 = PG // nsplit * sp_i
                        w1 = PG // nsplit * (sp_i + 1)
                        store_eng.dma_start(
                            y[img, :, PG * g + w0 : PG * g + w1, :],
                            sb[:, w0:w1],
                        )

    if wouter or same_w:
        _dedup_ldweights(nc)
    nc.compile()
    return nc


def _f8(a):
    import ml_dtypes

    return a.astype(ml_dtypes.float8_e4m3fn)


def _prep_inputs(x, Wf, b, use_fp8=USE_FP8, structural=STRUCT):
    n = x.shape[0]
    if structural:
        npdt = mybir.dt.np(F8E3)
        pl = np.asarray(x, np.float32).astype(npdt)
        # xi[i, (r c), q, j, w]: q=0 -> row-pair j (rows 2j-1, 2j), q=1 ->
        # row-pair j+1; borders zero
        xi = np.zeros((n, 2, C, 2, NPB, WPX), dtype=npdt)
        xi[:, 0, :, 0, 1:113, 1:225] = pl[:, 1:224:2].transpose(0, 3, 1, 2)
        xi[:, 1, :, 0, 0:112, 1:225] = pl[:, 0:224:2].transpose(0, 3, 1, 2)
        xi[:, :, :, 1, 0:113] = xi[:, :, :, 0, 1:114]
        xi = xi.reshape(n, 128, 2, NPB, WPX)

        Wq = np.sign(Wf).astype(np.float32)
        wm = np.zeros((128, 3, 2, 128), dtype=npdt)
        for dx in range(3):
            a = np.zeros((128, 128), dtype=np.float32)
            a[0:64, 0:64] = Wq[0, dx]
            a[64:128, 0:64] = Wq[1, dx]
            a[64:128, 64:128] = Wq[0, dx]
            bb = np.zeros((128, 128), dtype=np.float32)
            bb[0:64, 0:64] = Wq[2, dx]
            bb[0:64, 64:128] = Wq[1, dx]
            bb[64:128, 64:128] = Wq[2, dx]
            wm[:, dx, 0] = a.astype(npdt)
            wm[:, dx, 1] = bb.astype(npdt)
        bv = np.concatenate([b, b]).astype(np.float32).reshape(128, 1)
        return xi, wm, bv
    nplane = 2 if use_fp8 else 1
    if use_fp8:
        x = np.asarray(x, np.float32)
        hi = _f8(x)
        lo = _f8(x - hi.astype(np.float32))
        planes = [hi, lo]
        npdt = hi.dtype
    else:
        planes = [x.astype(np.float16)]
        npdt = np.float16

    # xi[i, (r c), q, j, 1+w] = plane_q[i, 2j+r-1, w, c]; border rows/cols 0
    xi = np.zeros((n, 2, C, nplane, NPB, WPX), dtype=npdt)
    for q, pl in enumerate(planes):
        # r=0: orig row 2j-1, j in 1..112 ; r=1: orig row 2j, j in 0..111
        xi[:, 0, :, q, 1:113, 1:225] = pl[:, 1:224:2].transpose(0, 3, 1, 2)
        xi[:, 1, :, q, 0:112, 1:225] = pl[:, 0:224:2].transpose(0, 3, 1, 2)
    xi = xi.reshape(n, 128, nplane, NPB, WPX)

    Wq = np.sign(Wf).astype(np.float32)  # [3(kh), 3(kw), 64(ci), 64(co)]
    wm = np.zeros((128, 6, nplane, 128), dtype=npdt)
    for dx in range(3):
        a = np.zeros((128, 128), dtype=np.float32)
        a[0:64, 0:64] = Wq[0, dx]
        a[64:128, 0:64] = Wq[1, dx]
        a[64:128, 64:128] = Wq[0, dx]
        bb = np.zeros((128, 128), dtype=np.float32)
        bb[0:64, 0:64] = Wq[2, dx]
        bb[0:64, 64:128] = Wq[1, dx]
        bb[64:128, 64:128] = Wq[2, dx]
        for q in range(nplane):
            wm[:, dx, q] = a.astype(npdt)
            wm[:, 3 + dx, q] = bb.astype(npdt)

    bv = np.concatenate([b, b]).astype(np.float32).reshape(128, 1)
    return xi, wm, bv


def _unpack_y(y_dev):
    # y_dev [n, 128, 112, 224] f16 -> [n, 224, 224, 64] f32
    n = y_dev.shape[0]
    v = y_dev.reshape(n, 2, 64, NPAIR, W)
    return (
        v.transpose(0, 3, 1, 4, 2).reshape(n, H, W, C).astype(np.float32)
    )


def kernel(x, W, b):
    global _cached
    if _cached is None:
        _cached = _build()
    nc = _cached

    xi, wm, bv = _prep_inputs(np.asarray(x), np.asarray(W), np.asarray(b))
    in_maps = [
        {"xi": np.ascontiguousarray(xi[IMG * core : IMG * (core + 1)]), "wm": wm, "bv": bv}
        for core in range(NCORES)
    ]
    trace = bool(os.environ.get("KERNEL_TRACE"))
    res = run_bass_kernel_spmd(nc, in_maps, core_ids=list(range(NCORES)), trace=trace)
    kernel.last_results = res
    out = np.concatenate([_unpack_y(r["y"]) for r in res.results], axis=0)
    return out

